# revision 2
# baseline (speedup 1.0000x reference)
"""Trainium2 Bass kernel for nn_ComplexAttention (B=16, L=500, D=1024, H=16).

Sharding: data-parallel over batch across 8 NeuronCores (2 batches/core).
All activations are kept feature-on-partition ("transposed" [D, L]) on device so
projections, scores, attention*V and the final linear need no on-device
transposes. Inputs are pre-transposed on the host; the output magnitude is
un-transposed on the host after the gather.
"""

import math

import numpy as np

import concourse.bacc as bacc
import concourse.mybir as mybir
import concourse.tile as tile
from concourse.bass_utils import run_bass_kernel_spmd

F32 = mybir.dt.float32
AF = mybir.ActivationFunctionType

B, L, D, H, DH = 16, 500, 1024, 16, 64
NCORES = 8
BP = B // NCORES          # batches per core
SCALE = 1.0 / math.sqrt(DH)
NT = D // 128             # 8 feature tiles of 128
KT = 4                    # key/query tiles of 125
KTS = L // KT             # 125
M = BP * L                # 1000 tokens per core

_CACHE = {}


def _build():
    nc = bacc.Bacc("TRN2", target_bir_lowering=False, debug=False)

    kT_d = nc.dram_tensor("kT", [BP, D, L], F32, kind="ExternalInput")
    vT_d = nc.dram_tensor("vT", [BP, D, L], F32, kind="ExternalInput")
    qT_d = nc.dram_tensor("qT", [BP, D, L], F32, kind="ExternalInput")
    phT_d = nc.dram_tensor("phT", [D, L], F32, kind="ExternalInput")
    Wk_d = nc.dram_tensor("Wk", [D, D], F32, kind="ExternalInput")
    Wv_d = nc.dram_tensor("Wv", [D, D], F32, kind="ExternalInput")
    Wq_d = nc.dram_tensor("Wq", [D, D], F32, kind="ExternalInput")
    Wf_d = nc.dram_tensor("Wf", [D, D], F32, kind="ExternalInput")
    bk_d = nc.dram_tensor("bk", [D], F32, kind="ExternalInput")
    bv_d = nc.dram_tensor("bv", [D], F32, kind="ExternalInput")
    bq_d = nc.dram_tensor("bq", [D], F32, kind="ExternalInput")
    bf_d = nc.dram_tensor("bf", [D], F32, kind="ExternalInput")

    outT_d = nc.dram_tensor("outT", [BP, D, L], F32, kind="ExternalOutput")
    ta_d = nc.dram_tensor("ta", [BP, L, L], F32, kind="ExternalOutput")

    with tile.TileContext(nc) as tc:
        _emit(nc, tc, locals())
    nc.compile()
    return nc


def _emit(nc, tc, t):
    from contextlib import ExitStack

    kT_d, vT_d, qT_d, phT_d = t["kT_d"], t["vT_d"], t["qT_d"], t["phT_d"]
    Wk_d, Wv_d, Wq_d, Wf_d = t["Wk_d"], t["Wv_d"], t["Wq_d"], t["Wf_d"]
    bk_d, bv_d, bq_d, bf_d = t["bk_d"], t["bv_d"], t["bq_d"], t["bf_d"]
    outT_d, ta_d = t["outT_d"], t["ta_d"]

    with ExitStack() as ctx:
        const = ctx.enter_context(tc.tile_pool(name="const", bufs=1))
        psp = ctx.enter_context(tc.tile_pool(name="ps", bufs=2, space="PSUM"))
        dram = ctx.enter_context(tc.tile_pool(name="dram", bufs=1, space="DRAM"))

        # ---- constants -------------------------------------------------
        ones_row = const.tile([1, KTS], F32, name="ones_row")
        nc.any.memset(ones_row[:], 1.0)
        bv_row = const.tile([1, D], F32, name="bv_row")
        nc.sync.dma_start(bv_row[:], bv_d.ap().rearrange("(one d) -> one d", one=1))
        bias_tiles = {}
        for nm, bd in (("bk", bk_d), ("bq", bq_d), ("bf", bf_d)):
            for dt in range(NT):
                bt = const.tile([128, 1], F32, name=f"{nm}_{dt}")
                nc.sync.dma_start(
                    bt[:],
                    bd.ap()[dt * 128:(dt + 1) * 128].rearrange("(p one) -> p one", one=1),
                )
                bias_tiles[(nm, dt)] = bt

        # ---- sin(phase), transposed layout ----------------------------
        sin_tiles = []
        with tc.tile_pool(name="phl", bufs=2) as phl:
            for dt in range(NT):
                ph_t = phl.tile([128, L], F32, tag="ph", name=f"ph_{dt}")
                nc.sync.dma_start(ph_t[:], phT_d.ap()[dt * 128:(dt + 1) * 128, :])
                s_t = const.tile([128, L], F32, name=f"sin_{dt}")
                nc.scalar.activation(s_t[:], ph_t[:], AF.Sin)
                sin_tiles.append(s_t)

        # ---- internal DRAM staging ------------------------------------
        qr_d = dram.tile([BP, D, L], F32, name="qr_st")
        qi_d = dram.tile([BP, D, L], F32, name="qi_st")
        kr_d = dram.tile([BP, D, L], F32, name="kr_st")
        ki_d = dram.tile([BP, D, L], F32, name="ki_st")
        vr_d = dram.tile([BP, L, D], F32, name="vr_st")
        vi_d = dram.tile([BP, L, D], F32, name="vi_st")

        # ---- phase B: projections -------------------------------------
        with ExitStack() as pctx:
            actp = pctx.enter_context(tc.tile_pool(name="actp", bufs=8))
            wp = pctx.enter_context(tc.tile_pool(name="wp", bufs=2))
            stp = pctx.enter_context(tc.tile_pool(name="stp", bufs=4))

            for src, (act_d, W_d, bnm, or_d, oi_d) in (
                ("q", (qT_d, Wq_d, "bq", qr_d, qi_d)),
                ("k", (kT_d, Wk_d, "bk", kr_d, ki_d)),
                ("v", (vT_d, Wv_d, None, vr_d, vi_d)),
            ):
                ar, ai = [], []
                for it in range(NT):
                    a_t = actp.tile([128, M], F32, tag="actR", name=f"a{src}_{it}")
                    for b in range(BP):
                        nc.sync.dma_start(
                            a_t[:, b * L:(b + 1) * L],
                            act_d.ap()[b, it * 128:(it + 1) * 128, :],
                        )
                    i_t = actp.tile([128, M], F32, tag="actI", name=f"ai{src}_{it}")
                    for b in range(BP):
                        nc.vector.tensor_mul(
                            i_t[:, b * L:(b + 1) * L],
                            a_t[:, b * L:(b + 1) * L],
                            sin_tiles[it][:],
                        )
                    ar.append(a_t)
                    ai.append(i_t)

                if src in ("q", "k"):
                    # transposed projection: out[dout, tok] = W.T @ actT
                    for dt in range(NT):
                        wts = []
                        for it in range(NT):
                            w_t = wp.tile([128, 128], F32, tag="wqk", bufs=16,
                                          name=f"w{src}_{dt}_{it}")
                            nc.sync.dma_start(
                                w_t[:],
                                W_d.ap()[it * 128:(it + 1) * 128, dt * 128:(dt + 1) * 128],
                            )
                            wts.append(w_t)
                        for acts, od in ((ar, or_d), (ai, oi_d)):
                            for b in range(BP):
                                ps_t = psp.tile([128, 512], F32, tag="proj",
                                                name=f"pp{src}_{dt}_{b}")
                                for it in range(NT):
                                    nc.tensor.matmul(
                                        ps_t[:, 0:L],
                                        wts[it][:],
                                        acts[it][:, b * L:(b + 1) * L],
                                        start=(it == 0), stop=(it == NT - 1),
                                    )
                                st_t = stp.tile([128, 512], F32, tag="stage",
                                                name=f"st{src}_{dt}_{b}")
                                nc.scalar.activation(
                                    st_t[:, 0:L], ps_t[:, 0:L], AF.Identity,
                                    bias=bias_tiles[(bnm, dt)][:],
                                )
                                nc.sync.dma_start(
                                    od[b, dt * 128:(dt + 1) * 128, :], st_t[:, 0:L]
                                )
                else:
                    # natural projection for V: out[tok, dout] (lhsT = actT tile)
                    for c in range(2):  # dout chunks of 512
                        wvs = []
                        for it in range(NT):
                            wv_t = wp.tile([128, 512], F32, tag="wv", bufs=10,
                                           name=f"wv_{c}_{it}")
                            nc.sync.dma_start(
                                wv_t[:],
                                W_d.ap()[it * 128:(it + 1) * 128, c * 512:(c + 1) * 512],
                            )
                            wvs.append(wv_t)
                        for acts, od in ((ar, vr_d), (ai, vi_d)):
                            for mt in range(2 * KT):  # token tiles of 125
                                ps_t = psp.tile([128, 512], F32, tag="proj",
                                                name=f"ppv_{c}_{mt}")
                                msl = slice(mt * KTS, (mt + 1) * KTS)
                                for it in range(NT):
                                    nc.tensor.matmul(
                                        ps_t[0:KTS, :], acts[it][:, msl], wvs[it][:],
                                        start=(it == 0), stop=False,
                                    )
                                nc.tensor.matmul(
                                    ps_t[0:KTS, :], ones_row[:],
                                    bv_row[:, c * 512:(c + 1) * 512],
                                    start=False, stop=True,
                                )
                                st_t = stp.tile([128, 512], F32, tag="stage",
                                                name=f"stv_{c}_{mt}")
                                nc.vector.tensor_copy(st_t[0:KTS, :], ps_t[0:KTS, :])
                                b, r = divmod(mt, KT)
                                nc.sync.dma_start(
                                    od[b, r * KTS:(r + 1) * KTS, c * 512:(c + 1) * 512],
                                    st_t[0:KTS, :],
                                )

        # ---- phase C: attention + final linear ------------------------
        with ExitStack() as actx:
            wfp = actx.enter_context(tc.tile_pool(name="wfp", bufs=64))
            hp = actx.enter_context(tc.tile_pool(name="hp", bufs=2))
            vp = actx.enter_context(tc.tile_pool(name="vp", bufs=10))
            ep = actx.enter_context(tc.tile_pool(name="ep", bufs=3))
            ip = actx.enter_context(tc.tile_pool(name="ip", bufs=3))
            ctxp = actx.enter_context(tc.tile_pool(name="ctxp", bufs=16))
            tap = actx.enter_context(tc.tile_pool(name="tap", bufs=10))
            fp = actx.enter_context(tc.tile_pool(name="fp", bufs=2))

            wf_tiles = {}
            for dt in range(NT):
                for it in range(NT):
                    wf_t = wfp.tile([128, 128], F32, tag="wf", name=f"wf_{dt}_{it}")
                    nc.sync.dma_start(
                        wf_t[:],
                        Wf_d.ap()[it * 128:(it + 1) * 128, dt * 128:(dt + 1) * 128],
                    )
                    wf_tiles[(dt, it)] = wf_t

            for b in range(BP):
                ctx_tiles = {0: [], 1: []}
                for ri in range(2):
                    for it in range(NT):
                        c_t = ctxp.tile([128, L], F32, tag="ctx",
                                        name=f"ctx{b}_{ri}_{it}")
                        ctx_tiles[ri].append(c_t)
                ta_saved = {0: [], 1: []}

                for h in range(H):
                    hsl = slice(h * DH, (h + 1) * DH)
                    kr = hp.tile([DH, L], F32, tag="kr", name=f"kr{b}_{h}")
                    nc.sync.dma_start(kr[:], kr_d[b, hsl, :])
                    ki = hp.tile([DH, L], F32, tag="ki", name=f"ki{b}_{h}")
                    nc.sync.dma_start(ki[:], ki_d[b, hsl, :])
                    qr = hp.tile([DH, L], F32, tag="qr", name=f"qr{b}_{h}")
                    nc.sync.dma_start(qr[:], qr_d[b, hsl, :])
                    qi = hp.tile([DH, L], F32, tag="qi", name=f"qi{b}_{h}")
                    nc.sync.dma_start(qi[:], qi_d[b, hsl, :])
                    qrn = hp.tile([DH, L], F32, tag="qrn", name=f"qrn{b}_{h}")
                    nc.vector.tensor_scalar_mul(qrn[:], qr[:], -1.0)

                    v1 = {0: [], 1: []}
                    for ri, vd in ((0, vr_d), (1, vi_d)):
                        for kt in range(KT):
                            v_t = vp.tile([KTS, DH + 1], F32, tag="v1",
                                          name=f"v1_{b}_{h}_{ri}_{kt}")
                            nc.sync.dma_start(
                                v_t[:, 0:DH],
                                vd[b, kt * KTS:(kt + 1) * KTS, hsl],
                            )
                            nc.any.memset(v_t[:, DH:DH + 1], 1.0)
                            v1[ri].append(v_t)

                    # main path: scores/attn transposed [keys, q]
                    for ri, (ka, kb, qa, qb) in enumerate(
                        (((kr, ki, qr, qi)), ((kr, ki, qi, qrn)))
                    ):
                        exp_tiles = []
                        for g in range(2):
                            sc = psp.tile([KTS, 1024], F32, tag="sc",
                                          name=f"sc{b}_{h}_{ri}_{g}")
                            for j in range(2):
                                kt = g * 2 + j
                                ksl = slice(kt * KTS, (kt + 1) * KTS)
                                off = j * 512
                                nc.tensor.matmul(sc[:, off:off + L], ka[:, ksl], qa[:],
                                                 start=True, stop=False)
                                nc.tensor.matmul(sc[:, off:off + L], kb[:, ksl], qb[:],
                                                 start=False, stop=True)
                            ex = ep.tile([KTS, 1012], F32, tag="exp",
                                         name=f"ex{b}_{h}_{ri}_{g}")
                            nc.scalar.activation(ex[:], sc[:, 0:1012], AF.Exp,
                                                 scale=SCALE)
                            exp_tiles.append(ex)
                        pv = psp.tile([DH + 1, 512], F32, tag="pv",
                                      name=f"pv{b}_{h}_{ri}")
                        for g in range(2):
                            for j in range(2):
                                kt = g * 2 + j
                                nc.tensor.matmul(
                                    pv[:, 0:L], v1[ri][kt][:],
                                    exp_tiles[g][:, j * 512:j * 512 + L],
                                    start=(kt == 0), stop=(kt == KT - 1),
                                )
                        invz = ip.tile([1, L], F32, tag="invz", name=f"iz{b}_{h}_{ri}")
                        nc.vector.reciprocal(invz[:], pv[DH:DH + 1, 0:L])
                        invzb = ip.tile([DH, L], F32, tag="invzb",
                                        name=f"izb{b}_{h}_{ri}")
                        nc.gpsimd.partition_broadcast(invzb[:], invz[:])
                        po = (h % 2) * DH
                        nc.vector.tensor_mul(
                            ctx_tiles[ri][h // 2][po:po + DH, :],
                            pv[0:DH, 0:L], invzb[:],
                        )

                    if h == 0:
                        # top_attn path, natural [q, keys] orientation
                        for ri, (la, lb, ra, rb) in enumerate(
                            (((qr, qi, kr, ki)), ((qi, qrn, kr, ki)))
                        ):
                            for qt in range(KT):
                                s0 = psp.tile([KTS, 512], F32, tag="pv",
                                              name=f"s0_{b}_{ri}_{qt}")
                                qsl = slice(qt * KTS, (qt + 1) * KTS)
                                nc.tensor.matmul(s0[:, 0:L], la[:, qsl], ra[:],
                                                 start=True, stop=False)
                                nc.tensor.matmul(s0[:, 0:L], lb[:, qsl], rb[:],
                                                 start=False, stop=True)
                                e0 = ip.tile([KTS, L], F32, tag="e0", bufs=4,
                                             name=f"e0_{b}_{ri}_{qt}")
                                z0 = ip.tile([KTS, 1], F32, tag="z0", bufs=4,
                                             name=f"z0_{b}_{ri}_{qt}")
                                nc.scalar.activation(e0[:], s0[:, 0:L], AF.Exp,
                                                     scale=SCALE, accum_out=z0[:])
                                iz0 = ip.tile([KTS, 1], F32, tag="iz0", bufs=4,
                                              name=f"iz0_{b}_{ri}_{qt}")
                                nc.vector.reciprocal(iz0[:], z0[:])
                                a0 = tap.tile([KTS, L], F32, tag="ta",
                                              name=f"a0_{b}_{ri}_{qt}")
                                nc.vector.tensor_scalar_mul(a0[:], e0[:], iz0[:])
                                ta_saved[ri].append(a0)

                # final linear (per dout tile) + magnitude
                for dt in range(NT):
                    fo = {}
                    for ri in range(2):
                        ps_f = psp.tile([128, 512], F32, tag="proj",
                                        name=f"pf{b}_{ri}_{dt}")
                        for it in range(NT):
                            nc.tensor.matmul(
                                ps_f[:, 0:L], wf_tiles[(dt, it)][:],
                                ctx_tiles[ri][it][:],
                                start=(it == 0), stop=(it == NT - 1),
                            )
                        f_t = fp.tile([128, L], F32, tag=f"fo{ri}",
                                      name=f"fo{b}_{ri}_{dt}")
                        nc.scalar.activation(f_t[:], ps_f[:, 0:L], AF.Identity,
                                             bias=bias_tiles[("bf", dt)][:])
                        fo[ri] = f_t
                    m1 = fp.tile([128, L], F32, tag="m1", name=f"m1_{b}_{dt}")
                    nc.vector.tensor_mul(m1[:], fo[0][:], fo[0][:])
                    m2 = fp.tile([128, L], F32, tag="m2", name=f"m2_{b}_{dt}")
                    nc.vector.tensor_mul(m2[:], fo[1][:], fo[1][:])
                    nc.vector.tensor_add(m1[:], m1[:], m2[:])
                    mo = fp.tile([128, L], F32, tag="mo", name=f"mo_{b}_{dt}")
                    nc.scalar.activation(mo[:], m1[:], AF.Sqrt)
                    nc.sync.dma_start(outT_d.ap()[b, dt * 128:(dt + 1) * 128, :], mo[:])

                # top_attn magnitude
                for qt in range(KT):
                    a_r, a_i = ta_saved[0][qt], ta_saved[1][qt]
                    t1 = fp.tile([KTS, L], F32, tag="t1", name=f"t1_{b}_{qt}")
                    nc.vector.tensor_mul(t1[:], a_r[:], a_r[:])
                    t2 = fp.tile([KTS, L], F32, tag="t2", name=f"t2_{b}_{qt}")
                    nc.vector.tensor_mul(t2[:], a_i[:], a_i[:])
                    nc.vector.tensor_add(t1[:], t1[:], t2[:])
                    to = fp.tile([KTS, L], F32, tag="to", name=f"to_{b}_{qt}")
                    nc.scalar.activation(to[:], t1[:], AF.Sqrt)
                    nc.sync.dma_start(ta_d.ap()[b, qt * KTS:(qt + 1) * KTS, :], to[:])


def _get_module():
    if "nc" not in _CACHE:
        _CACHE["nc"] = _build()
    return _CACHE["nc"]


def _run(inputs, trace=False):
    nc = _get_module()
    key = np.asarray(inputs["key"], np.float32)
    value = np.asarray(inputs["value"], np.float32)
    query = np.asarray(inputs["query"], np.float32)
    phT = np.ascontiguousarray(np.asarray(inputs["phase"], np.float32).T)
    Ws = {n: np.ascontiguousarray(np.asarray(inputs[n], np.float32))
          for n in ("Wk", "Wv", "Wq", "Wf")}
    bs = {n: np.asarray(inputs[n], np.float32) for n in ("bk", "bv", "bq", "bf")}

    in_maps = []
    for c in range(NCORES):
        sl = slice(c * BP, (c + 1) * BP)
        in_maps.append({
            "kT": np.ascontiguousarray(key[sl].transpose(0, 2, 1)),
            "vT": np.ascontiguousarray(value[sl].transpose(0, 2, 1)),
            "qT": np.ascontiguousarray(query[sl].transpose(0, 2, 1)),
            "phT": phT,
            "Wk": Ws["Wk"], "Wv": Ws["Wv"], "Wq": Ws["Wq"], "Wf": Ws["Wf"],
            "bk": bs["bk"], "bv": bs["bv"], "bq": bs["bq"], "bf": bs["bf"],
        })

    res = run_bass_kernel_spmd(nc, in_maps, list(range(NCORES)), trace=trace)
    out = np.empty((B, L, D), np.float32)
    ta = np.empty((B, L, L), np.float32)
    for c in range(NCORES):
        sl = slice(c * BP, (c + 1) * BP)
        out[sl] = res.results[c]["outT"].transpose(0, 2, 1)
        ta[sl] = res.results[c]["ta"]
    return (out, ta), res


def kernel(**inputs):
    (out, ta), _ = _run(inputs, trace=False)
    return out, ta


# revision 5
# speedup vs baseline: 2.0998x; 2.0998x over previous
"""Trainium2 Bass kernel for nn_ComplexAttention (B=16, L=500, D=1024, H=16).

Sharding: data-parallel over batch across 8 NeuronCores (2 batches/core).
All activations are kept feature-on-partition ("transposed" [D, L]) on device so
projections, scores, attention*V and the final linear need no on-device
transposes. Inputs are pre-transposed on the host; the output magnitude is
un-transposed on the host after the gather.
"""

import math

import numpy as np

import concourse.bacc as bacc
import concourse.mybir as mybir
import concourse.tile as tile
from concourse.bass_utils import run_bass_kernel_spmd

F32 = mybir.dt.float32
F32R = mybir.dt.float32r
AF = mybir.ActivationFunctionType

B, L, D, H, DH = 16, 500, 1024, 16, 64
NCORES = 8
BP = B // NCORES          # batches per core
SCALE = 1.0 / math.sqrt(DH)
NT = D // 128             # 8 feature tiles of 128
KT = 4                    # key/query tiles of 125
KTS = L // KT             # 125
M = BP * L                # 1000 tokens per core

_CACHE = {}


def _build():
    nc = bacc.Bacc("TRN2", target_bir_lowering=False, debug=False)

    kT_d = nc.dram_tensor("kT", [BP, D, L], F32R, kind="ExternalInput")
    vT_d = nc.dram_tensor("vT", [BP, D, L], F32R, kind="ExternalInput")
    qT_d = nc.dram_tensor("qT", [BP, D, L], F32R, kind="ExternalInput")
    phT_d = nc.dram_tensor("phT", [D, L], F32, kind="ExternalInput")
    Wk_d = nc.dram_tensor("Wk", [D, D], F32R, kind="ExternalInput")
    Wv_d = nc.dram_tensor("Wv", [D, D], F32R, kind="ExternalInput")
    Wq_d = nc.dram_tensor("Wq", [D, D], F32R, kind="ExternalInput")
    Wf_d = nc.dram_tensor("Wf", [D, D], F32R, kind="ExternalInput")
    bk_d = nc.dram_tensor("bk", [D], F32, kind="ExternalInput")
    bv_d = nc.dram_tensor("bv", [D], F32, kind="ExternalInput")
    bq_d = nc.dram_tensor("bq", [D], F32, kind="ExternalInput")
    bf_d = nc.dram_tensor("bf", [D], F32, kind="ExternalInput")

    outT_d = nc.dram_tensor("outT", [BP, D, L], F32, kind="ExternalOutput")
    ta_d = nc.dram_tensor("ta", [BP, L, L], F32, kind="ExternalOutput")

    with tile.TileContext(nc) as tc:
        _emit(nc, tc, locals())
    nc.compile()
    return nc


def _emit(nc, tc, t):
    from contextlib import ExitStack

    kT_d, vT_d, qT_d, phT_d = t["kT_d"], t["vT_d"], t["qT_d"], t["phT_d"]
    Wk_d, Wv_d, Wq_d, Wf_d = t["Wk_d"], t["Wv_d"], t["Wq_d"], t["Wf_d"]
    bk_d, bv_d, bq_d, bf_d = t["bk_d"], t["bv_d"], t["bq_d"], t["bf_d"]
    outT_d, ta_d = t["outT_d"], t["ta_d"]

    with ExitStack() as ctx:
        const = ctx.enter_context(tc.tile_pool(name="const", bufs=1))
        psp = ctx.enter_context(tc.tile_pool(name="ps", bufs=2, space="PSUM"))
        dram = ctx.enter_context(tc.tile_pool(name="dram", bufs=1, space="DRAM"))

        # ---- constants -------------------------------------------------
        ones_row = const.tile([1, KTS], F32R, name="ones_row")
        nc.any.memset(ones_row[:].bitcast(F32), 1.0)
        bv_row = const.tile([1, D], F32R, name="bv_row")
        nc.sync.dma_start(
            bv_row[:],
            bv_d.ap().rearrange("(one d) -> one d", one=1).bitcast(F32R),
        )
        bias_tiles = {}
        for nm, bd in (("bk", bk_d), ("bq", bq_d), ("bf", bf_d)):
            for dt in range(NT):
                bt = const.tile([128, 1], F32, name=f"{nm}_{dt}")
                nc.sync.dma_start(
                    bt[:],
                    bd.ap()[dt * 128:(dt + 1) * 128].rearrange("(p one) -> p one", one=1),
                )
                bias_tiles[(nm, dt)] = bt

        # ---- sin(phase), transposed layout ----------------------------
        sin_tiles = []
        with tc.tile_pool(name="phl", bufs=2) as phl:
            for dt in range(NT):
                ph_t = phl.tile([128, L], F32, tag="ph", name=f"ph_{dt}")
                nc.sync.dma_start(ph_t[:], phT_d.ap()[dt * 128:(dt + 1) * 128, :])
                s_t = const.tile([128, L], F32, name=f"sin_{dt}")
                nc.scalar.activation(s_t[:], ph_t[:], AF.Sin)
                sin_tiles.append(s_t)

        # ---- internal DRAM staging ------------------------------------
        qr_d = dram.tile([BP, D, L], F32R, name="qr_st")
        qi_d = dram.tile([BP, D, L], F32R, name="qi_st")
        kr_d = dram.tile([BP, D, L], F32R, name="kr_st")
        ki_d = dram.tile([BP, D, L], F32R, name="ki_st")
        vr_d = dram.tile([BP, L, D], F32R, name="vr_st")
        vi_d = dram.tile([BP, L, D], F32R, name="vi_st")

        # ---- phase B: projections -------------------------------------
        with ExitStack() as pctx:
            actp = pctx.enter_context(tc.tile_pool(name="actp", bufs=8))
            wp = pctx.enter_context(tc.tile_pool(name="wp", bufs=2))
            stp = pctx.enter_context(tc.tile_pool(name="stp", bufs=4))

            for src, (act_d, W_d, bnm, or_d, oi_d) in (
                ("q", (qT_d, Wq_d, "bq", qr_d, qi_d)),
                ("k", (kT_d, Wk_d, "bk", kr_d, ki_d)),
                ("v", (vT_d, Wv_d, None, vr_d, vi_d)),
            ):
                ar, ai = [], []
                for it in range(NT):
                    a_t = actp.tile([128, M], F32R, tag="actR", name=f"a{src}_{it}")
                    for b in range(BP):
                        nc.sync.dma_start(
                            a_t[:, b * L:(b + 1) * L],
                            act_d.ap()[b, it * 128:(it + 1) * 128, :],
                        )
                    i_t = actp.tile([128, M], F32R, tag="actI", name=f"ai{src}_{it}")
                    for b in range(BP):
                        nc.vector.tensor_mul(
                            i_t[:, b * L:(b + 1) * L],
                            a_t[:, b * L:(b + 1) * L].bitcast(F32),
                            sin_tiles[it][:],
                        )
                    ar.append(a_t)
                    ai.append(i_t)

                if src in ("q", "k"):
                    # transposed projection: out[dout, tok] = W.T @ actT
                    for dt in range(NT):
                        wts = []
                        for it in range(NT):
                            w_t = wp.tile([128, 128], F32R, tag="wqk", bufs=16,
                                          name=f"w{src}_{dt}_{it}")
                            nc.sync.dma_start(
                                w_t[:],
                                W_d.ap()[it * 128:(it + 1) * 128, dt * 128:(dt + 1) * 128],
                            )
                            wts.append(w_t)
                        for acts, od in ((ar, or_d), (ai, oi_d)):
                            for b in range(BP):
                                ps_t = psp.tile([128, 512], F32, tag="proj",
                                                name=f"pp{src}_{dt}_{b}")
                                for it in range(NT):
                                    nc.tensor.matmul(
                                        ps_t[:, 0:L],
                                        wts[it][:],
                                        acts[it][:, b * L:(b + 1) * L],
                                        start=(it == 0), stop=(it == NT - 1),
                                    )
                                st_t = stp.tile([128, 512], F32R, tag="stage",
                                                name=f"st{src}_{dt}_{b}")
                                nc.scalar.activation(
                                    st_t[:, 0:L], ps_t[:, 0:L], AF.Identity,
                                    bias=bias_tiles[(bnm, dt)][:],
                                )
                                nc.sync.dma_start(
                                    od[b, dt * 128:(dt + 1) * 128, :], st_t[:, 0:L]
                                )
                else:
                    # natural projection for V: out[tok, dout] (lhsT = actT tile)
                    for c in range(2):  # dout chunks of 512
                        wvs = []
                        for it in range(NT):
                            wv_t = wp.tile([128, 512], F32R, tag="wv", bufs=10,
                                           name=f"wv_{c}_{it}")
                            nc.sync.dma_start(
                                wv_t[:],
                                W_d.ap()[it * 128:(it + 1) * 128, c * 512:(c + 1) * 512],
                            )
                            wvs.append(wv_t)
                        for acts, od in ((ar, vr_d), (ai, vi_d)):
                            for mt in range(2 * KT):  # token tiles of 125
                                ps_t = psp.tile([128, 512], F32, tag="proj",
                                                name=f"ppv_{c}_{mt}")
                                msl = slice(mt * KTS, (mt + 1) * KTS)
                                for it in range(NT):
                                    nc.tensor.matmul(
                                        ps_t[0:KTS, :], acts[it][:, msl], wvs[it][:],
                                        start=(it == 0), stop=False,
                                    )
                                nc.tensor.matmul(
                                    ps_t[0:KTS, :], ones_row[:],
                                    bv_row[:, c * 512:(c + 1) * 512],
                                    start=False, stop=True,
                                )
                                st_t = stp.tile([128, 512], F32R, tag="stage",
                                                name=f"stv_{c}_{mt}")
                                nc.vector.tensor_copy(st_t[0:KTS, :], ps_t[0:KTS, :])
                                b, r = divmod(mt, KT)
                                nc.sync.dma_start(
                                    od[b, r * KTS:(r + 1) * KTS, c * 512:(c + 1) * 512],
                                    st_t[0:KTS, :],
                                )

        # ---- phase C: attention + final linear ------------------------
        with ExitStack() as actx:
            wfp = actx.enter_context(tc.tile_pool(name="wfp", bufs=64))
            hp = actx.enter_context(tc.tile_pool(name="hp", bufs=2))
            vp = actx.enter_context(tc.tile_pool(name="vp", bufs=10))
            ep = actx.enter_context(tc.tile_pool(name="ep", bufs=3))
            ip = actx.enter_context(tc.tile_pool(name="ip", bufs=3))
            ctxp = actx.enter_context(tc.tile_pool(name="ctxp", bufs=16))
            tap = actx.enter_context(tc.tile_pool(name="tap", bufs=10))
            fp = actx.enter_context(tc.tile_pool(name="fp", bufs=2))

            wf_tiles = {}
            for dt in range(NT):
                for it in range(NT):
                    wf_t = wfp.tile([128, 128], F32R, tag="wf", name=f"wf_{dt}_{it}")
                    nc.sync.dma_start(
                        wf_t[:],
                        Wf_d.ap()[it * 128:(it + 1) * 128, dt * 128:(dt + 1) * 128],
                    )
                    wf_tiles[(dt, it)] = wf_t

            for b in range(BP):
                ctx_tiles = {0: [], 1: []}
                for ri in range(2):
                    for it in range(NT):
                        c_t = ctxp.tile([128, L], F32R, tag="ctx",
                                        name=f"ctx{b}_{ri}_{it}")
                        ctx_tiles[ri].append(c_t)
                ta_saved = {0: [], 1: []}

                for h in range(H):
                    hsl = slice(h * DH, (h + 1) * DH)
                    kr = hp.tile([DH, L], F32R, tag="kr", name=f"kr{b}_{h}")
                    nc.sync.dma_start(kr[:], kr_d[b, hsl, :])
                    ki = hp.tile([DH, L], F32R, tag="ki", name=f"ki{b}_{h}")
                    nc.sync.dma_start(ki[:], ki_d[b, hsl, :])
                    qr = hp.tile([DH, L], F32R, tag="qr", name=f"qr{b}_{h}")
                    nc.sync.dma_start(qr[:], qr_d[b, hsl, :])
                    qi = hp.tile([DH, L], F32R, tag="qi", name=f"qi{b}_{h}")
                    nc.sync.dma_start(qi[:], qi_d[b, hsl, :])
                    qrn = hp.tile([DH, L], F32R, tag="qrn", name=f"qrn{b}_{h}")
                    nc.vector.tensor_scalar_mul(qrn[:], qr[:].bitcast(F32), -1.0)

                    v1 = {0: [], 1: []}
                    for ri, vd in ((0, vr_d), (1, vi_d)):
                        for kt in range(KT):
                            v_t = vp.tile([KTS, DH + 1], F32R, tag="v1",
                                          name=f"v1_{b}_{h}_{ri}_{kt}")
                            nc.sync.dma_start(
                                v_t[:, 0:DH],
                                vd[b, kt * KTS:(kt + 1) * KTS, hsl],
                            )
                            nc.any.memset(v_t[:, DH:DH + 1].bitcast(F32), 1.0)
                            v1[ri].append(v_t)

                    # main path: scores/attn transposed [keys, q]
                    for ri, (ka, kb, qa, qb) in enumerate(
                        (((kr, ki, qr, qi)), ((kr, ki, qi, qrn)))
                    ):
                        exp_tiles = []
                        for g in range(2):
                            sc = psp.tile([KTS, 1024], F32, tag="sc",
                                          name=f"sc{b}_{h}_{ri}_{g}")
                            for j in range(2):
                                kt = g * 2 + j
                                ksl = slice(kt * KTS, (kt + 1) * KTS)
                                off = j * 512
                                nc.tensor.matmul(sc[:, off:off + L], ka[:, ksl], qa[:],
                                                 start=True, stop=False)
                                nc.tensor.matmul(sc[:, off:off + L], kb[:, ksl], qb[:],
                                                 start=False, stop=True)
                            ex = ep.tile([KTS, 1012], F32R, tag="exp",
                                         name=f"ex{b}_{h}_{ri}_{g}")
                            nc.scalar.activation(ex[:], sc[:, 0:1012], AF.Exp,
                                                 scale=SCALE)
                            exp_tiles.append(ex)
                        pv = psp.tile([DH + 1, 512], F32, tag="pv",
                                      name=f"pv{b}_{h}_{ri}")
                        for g in range(2):
                            for j in range(2):
                                kt = g * 2 + j
                                nc.tensor.matmul(
                                    pv[:, 0:L], v1[ri][kt][:],
                                    exp_tiles[g][:, j * 512:j * 512 + L],
                                    start=(kt == 0), stop=(kt == KT - 1),
                                )
                        invz = ip.tile([1, L], F32, tag="invz", name=f"iz{b}_{h}_{ri}")
                        nc.vector.reciprocal_approx_fast(invz[:], pv[DH:DH + 1, 0:L])
                        invzb = ip.tile([DH, L], F32, tag="invzb",
                                        name=f"izb{b}_{h}_{ri}")
                        nc.gpsimd.partition_broadcast(invzb[:], invz[:])
                        po = (h % 2) * DH
                        nc.vector.tensor_mul(
                            ctx_tiles[ri][h // 2][po:po + DH, :],
                            pv[0:DH, 0:L], invzb[:],
                        )

                    if h == 0:
                        # top_attn path, natural [q, keys] orientation
                        for ri, (la, lb, ra, rb) in enumerate(
                            (((qr, qi, kr, ki)), ((qi, qrn, kr, ki)))
                        ):
                            for qt in range(KT):
                                s0 = psp.tile([KTS, 512], F32, tag="pv",
                                              name=f"s0_{b}_{ri}_{qt}")
                                qsl = slice(qt * KTS, (qt + 1) * KTS)
                                nc.tensor.matmul(s0[:, 0:L], la[:, qsl], ra[:],
                                                 start=True, stop=False)
                                nc.tensor.matmul(s0[:, 0:L], lb[:, qsl], rb[:],
                                                 start=False, stop=True)
                                e0 = ip.tile([KTS, L], F32, tag="e0", bufs=4,
                                             name=f"e0_{b}_{ri}_{qt}")
                                z0 = ip.tile([KTS, 1], F32, tag="z0", bufs=4,
                                             name=f"z0_{b}_{ri}_{qt}")
                                nc.scalar.activation(e0[:], s0[:, 0:L], AF.Exp,
                                                     scale=SCALE, accum_out=z0[:])
                                iz0 = ip.tile([KTS, 1], F32, tag="iz0", bufs=4,
                                              name=f"iz0_{b}_{ri}_{qt}")
                                nc.vector.reciprocal_approx_fast(iz0[:], z0[:])
                                a0 = tap.tile([KTS, L], F32, tag="ta",
                                              name=f"a0_{b}_{ri}_{qt}")
                                nc.vector.tensor_scalar_mul(a0[:], e0[:], iz0[:])
                                ta_saved[ri].append(a0)

                # final linear (per dout tile) + magnitude
                for dt in range(NT):
                    fo = {}
                    for ri in range(2):
                        ps_f = psp.tile([128, 512], F32, tag="proj",
                                        name=f"pf{b}_{ri}_{dt}")
                        for it in range(NT):
                            nc.tensor.matmul(
                                ps_f[:, 0:L], wf_tiles[(dt, it)][:],
                                ctx_tiles[ri][it][:],
                                start=(it == 0), stop=(it == NT - 1),
                            )
                        f_t = fp.tile([128, L], F32, tag=f"fo{ri}",
                                      name=f"fo{b}_{ri}_{dt}")
                        nc.scalar.activation(f_t[:], ps_f[:, 0:L], AF.Identity,
                                             bias=bias_tiles[("bf", dt)][:])
                        fo[ri] = f_t
                    m1 = fp.tile([128, L], F32, tag="m1", name=f"m1_{b}_{dt}")
                    nc.vector.tensor_mul(m1[:], fo[0][:], fo[0][:])
                    m2 = fp.tile([128, L], F32, tag="m2", name=f"m2_{b}_{dt}")
                    nc.vector.tensor_mul(m2[:], fo[1][:], fo[1][:])
                    nc.vector.tensor_add(m1[:], m1[:], m2[:])
                    mo = fp.tile([128, L], F32, tag="mo", name=f"mo_{b}_{dt}")
                    nc.scalar.activation(mo[:], m1[:], AF.Sqrt)
                    nc.sync.dma_start(outT_d.ap()[b, dt * 128:(dt + 1) * 128, :], mo[:])

                # top_attn magnitude
                for qt in range(KT):
                    a_r, a_i = ta_saved[0][qt], ta_saved[1][qt]
                    t1 = fp.tile([KTS, L], F32, tag="t1", name=f"t1_{b}_{qt}")
                    nc.vector.tensor_mul(t1[:], a_r[:], a_r[:])
                    t2 = fp.tile([KTS, L], F32, tag="t2", name=f"t2_{b}_{qt}")
                    nc.vector.tensor_mul(t2[:], a_i[:], a_i[:])
                    nc.vector.tensor_add(t1[:], t1[:], t2[:])
                    to = fp.tile([KTS, L], F32, tag="to", name=f"to_{b}_{qt}")
                    nc.scalar.activation(to[:], t1[:], AF.Sqrt)
                    nc.sync.dma_start(ta_d.ap()[b, qt * KTS:(qt + 1) * KTS, :], to[:])


def _get_module():
    if "nc" not in _CACHE:
        _CACHE["nc"] = _build()
    return _CACHE["nc"]


def _run(inputs, trace=False):
    nc = _get_module()
    key = np.asarray(inputs["key"], np.float32)
    value = np.asarray(inputs["value"], np.float32)
    query = np.asarray(inputs["query"], np.float32)
    phT = np.ascontiguousarray(np.asarray(inputs["phase"], np.float32).T)
    Ws = {n: np.ascontiguousarray(np.asarray(inputs[n], np.float32))
          for n in ("Wk", "Wv", "Wq", "Wf")}
    bs = {n: np.asarray(inputs[n], np.float32) for n in ("bk", "bv", "bq", "bf")}

    in_maps = []
    for c in range(NCORES):
        sl = slice(c * BP, (c + 1) * BP)
        in_maps.append({
            "kT": np.ascontiguousarray(key[sl].transpose(0, 2, 1)),
            "vT": np.ascontiguousarray(value[sl].transpose(0, 2, 1)),
            "qT": np.ascontiguousarray(query[sl].transpose(0, 2, 1)),
            "phT": phT,
            "Wk": Ws["Wk"], "Wv": Ws["Wv"], "Wq": Ws["Wq"], "Wf": Ws["Wf"],
            "bk": bs["bk"], "bv": bs["bv"], "bq": bs["bq"], "bf": bs["bf"],
        })

    res = run_bass_kernel_spmd(nc, in_maps, list(range(NCORES)), trace=trace)
    out = np.empty((B, L, D), np.float32)
    ta = np.empty((B, L, L), np.float32)
    for c in range(NCORES):
        sl = slice(c * BP, (c + 1) * BP)
        out[sl] = res.results[c]["outT"].transpose(0, 2, 1)
        ta[sl] = res.results[c]["ta"]
    return (out, ta), res


def kernel(**inputs):
    (out, ta), _ = _run(inputs, trace=False)
    return out, ta


# revision 7
# speedup vs baseline: 2.8713x; 1.3674x over previous
"""Trainium2 Bass kernel for nn_ComplexAttention (B=16, L=500, D=1024, H=16).

Sharding: data-parallel over batch across 8 NeuronCores (2 batches/core).
All activations are kept feature-on-partition ("transposed" [D, L]) on device so
projections, scores, attention*V and the final linear need no on-device
transposes. Inputs are pre-transposed on the host; the output magnitude is
un-transposed on the host after the gather.

Precision: projections run in float32r (fp32 storage, ~1e-4 matmul rounding,
full PE rate). Attention-level matmuls (scores/PV/final) run in bf16 with fp32
PSUM accumulation; softmax statistics stay fp32.

Heads are processed in pairs: the two 64-row score matmuls occupy disjoint PE
row groups (partitions 0:64 / 64:128) and run concurrently.
"""

import math

import ml_dtypes
import numpy as np

import concourse.bacc as bacc
import concourse.mybir as mybir
import concourse.tile as tile
from concourse.bass_utils import run_bass_kernel_spmd

F32 = mybir.dt.float32
F32R = mybir.dt.float32r
BF16 = mybir.dt.bfloat16
AF = mybir.ActivationFunctionType

B, L, D, H, DH = 16, 500, 1024, 16, 64
NCORES = 8
BP = B // NCORES          # batches per core
SCALE = 1.0 / math.sqrt(DH)
NT = D // 128             # 8 feature tiles of 128
KT = 4                    # key/query tiles of 125
KTS = L // KT             # 125
M = BP * L                # 1000 tokens per core

_CACHE = {}


def _build():
    nc = bacc.Bacc("TRN2", target_bir_lowering=False, debug=False)

    kT_d = nc.dram_tensor("kT", [BP, D, L], F32R, kind="ExternalInput")
    vT_d = nc.dram_tensor("vT", [BP, D, L], F32R, kind="ExternalInput")
    qT_d = nc.dram_tensor("qT", [BP, D, L], F32R, kind="ExternalInput")
    phT_d = nc.dram_tensor("phT", [D, L], F32, kind="ExternalInput")
    Wk_d = nc.dram_tensor("Wk", [D, D], F32R, kind="ExternalInput")
    Wv_d = nc.dram_tensor("Wv", [D, D], F32R, kind="ExternalInput")
    Wq_d = nc.dram_tensor("Wq", [D, D], F32R, kind="ExternalInput")
    Wf_d = nc.dram_tensor("Wf", [D, D], BF16, kind="ExternalInput")
    bk_d = nc.dram_tensor("bk", [D], F32, kind="ExternalInput")
    bv_d = nc.dram_tensor("bv", [D], F32, kind="ExternalInput")
    bq_d = nc.dram_tensor("bq", [D], F32, kind="ExternalInput")
    bf_d = nc.dram_tensor("bf", [D], F32, kind="ExternalInput")

    outT_d = nc.dram_tensor("outT", [BP, D, L], F32, kind="ExternalOutput")
    ta_d = nc.dram_tensor("ta", [BP, L, L], F32, kind="ExternalOutput")

    with tile.TileContext(nc) as tc:
        _emit(nc, tc, locals())
    nc.compile()
    return nc


def _emit(nc, tc, t):
    from contextlib import ExitStack

    kT_d, vT_d, qT_d, phT_d = t["kT_d"], t["vT_d"], t["qT_d"], t["phT_d"]
    Wk_d, Wv_d, Wq_d, Wf_d = t["Wk_d"], t["Wv_d"], t["Wq_d"], t["Wf_d"]
    bk_d, bv_d, bq_d, bf_d = t["bk_d"], t["bv_d"], t["bq_d"], t["bf_d"]
    outT_d, ta_d = t["outT_d"], t["ta_d"]

    with ExitStack() as ctx:
        const = ctx.enter_context(tc.tile_pool(name="const", bufs=1))
        psp = ctx.enter_context(tc.tile_pool(name="ps", bufs=2, space="PSUM"))
        dram = ctx.enter_context(tc.tile_pool(name="dram", bufs=1, space="DRAM"))

        # ---- constants -------------------------------------------------
        ones_row = const.tile([1, KTS], F32R, name="ones_row")
        nc.any.memset(ones_row[:].bitcast(F32), 1.0)
        bv_row = const.tile([1, D], F32R, name="bv_row")
        nc.sync.dma_start(
            bv_row[:],
            bv_d.ap().rearrange("(one d) -> one d", one=1).bitcast(F32R),
        )
        bias_tiles = {}
        for nm, bd in (("bk", bk_d), ("bq", bq_d), ("bf", bf_d)):
            for dt in range(NT):
                bt = const.tile([128, 1], F32, name=f"{nm}_{dt}")
                nc.sync.dma_start(
                    bt[:],
                    bd.ap()[dt * 128:(dt + 1) * 128].rearrange("(p one) -> p one", one=1),
                )
                bias_tiles[(nm, dt)] = bt

        # ---- sin(phase), transposed layout ----------------------------
        sin_tiles = []
        with tc.tile_pool(name="phl", bufs=2) as phl:
            for dt in range(NT):
                ph_t = phl.tile([128, L], F32, tag="ph", name=f"ph_{dt}")
                nc.sync.dma_start(ph_t[:], phT_d.ap()[dt * 128:(dt + 1) * 128, :])
                s_t = const.tile([128, L], F32, name=f"sin_{dt}")
                nc.scalar.activation(s_t[:], ph_t[:], AF.Sin)
                sin_tiles.append(s_t)

        # ---- internal DRAM staging ------------------------------------
        qr_d = dram.tile([BP, D, L], BF16, name="qr_st")
        qi_d = dram.tile([BP, D, L], BF16, name="qi_st")
        kr_d = dram.tile([BP, D, L], BF16, name="kr_st")
        ki_d = dram.tile([BP, D, L], BF16, name="ki_st")
        vr_d = dram.tile([BP, L, D], BF16, name="vr_st")
        vi_d = dram.tile([BP, L, D], BF16, name="vi_st")

        # ---- phase B: projections -------------------------------------
        with ExitStack() as pctx:
            actp = pctx.enter_context(tc.tile_pool(name="actp", bufs=8))
            wp = pctx.enter_context(tc.tile_pool(name="wp", bufs=2))
            stp = pctx.enter_context(tc.tile_pool(name="stp", bufs=4))

            for src, (act_d, W_d, bnm, or_d, oi_d) in (
                ("q", (qT_d, Wq_d, "bq", qr_d, qi_d)),
                ("k", (kT_d, Wk_d, "bk", kr_d, ki_d)),
                ("v", (vT_d, Wv_d, None, vr_d, vi_d)),
            ):
                ar, ai = [], []
                for it in range(NT):
                    a_t = actp.tile([128, M], F32R, tag="actR", name=f"a{src}_{it}")
                    nc.sync.dma_start(
                        a_t[:].rearrange("p (b l) -> p b l", b=BP),
                        act_d.ap()[:, it * 128:(it + 1) * 128, :]
                            .rearrange("b p l -> p b l"),
                    )
                    i_t = actp.tile([128, M], F32R, tag="actI", name=f"ai{src}_{it}")
                    for b in range(BP):
                        nc.vector.tensor_mul(
                            i_t[:, b * L:(b + 1) * L],
                            a_t[:, b * L:(b + 1) * L].bitcast(F32),
                            sin_tiles[it][:],
                        )
                    ar.append(a_t)
                    ai.append(i_t)

                if src in ("q", "k"):
                    # transposed projection: out[dout, tok] = W.T @ actT
                    wts = []
                    for it in range(NT):
                        w_t = wp.tile([128, D], F32R, tag="wqk", bufs=10,
                                      name=f"w{src}_{it}")
                        nc.sync.dma_start(
                            w_t[:], W_d.ap()[it * 128:(it + 1) * 128, :]
                        )
                        wts.append(w_t)
                    for dt in range(NT):
                        dsl = slice(dt * 128, (dt + 1) * 128)
                        for acts, od in ((ar, or_d), (ai, oi_d)):
                            for b in range(BP):
                                ps_t = psp.tile([128, 512], F32, tag="proj",
                                                name=f"pp{src}_{dt}_{b}")
                                for it in range(NT):
                                    nc.tensor.matmul(
                                        ps_t[:, 0:L],
                                        wts[it][:, dsl],
                                        acts[it][:, b * L:(b + 1) * L],
                                        start=(it == 0), stop=(it == NT - 1),
                                    )
                                st_t = stp.tile([128, 512], BF16, tag="stage",
                                                name=f"st{src}_{dt}_{b}")
                                nc.scalar.activation(
                                    st_t[:, 0:L], ps_t[:, 0:L], AF.Identity,
                                    bias=bias_tiles[(bnm, dt)][:],
                                )
                                nc.sync.dma_start(
                                    od[b, dsl, :], st_t[:, 0:L]
                                )
                else:
                    # natural projection for V: out[tok, dout] (lhsT = actT tile)
                    for c in range(2):  # dout chunks of 512
                        wvs = []
                        for it in range(NT):
                            wv_t = wp.tile([128, 512], F32R, tag="wv", bufs=10,
                                           name=f"wv_{c}_{it}")
                            nc.sync.dma_start(
                                wv_t[:],
                                W_d.ap()[it * 128:(it + 1) * 128, c * 512:(c + 1) * 512],
                            )
                            wvs.append(wv_t)
                        for acts, od in ((ar, vr_d), (ai, vi_d)):
                            for mt in range(2 * KT):  # token tiles of 125
                                ps_t = psp.tile([128, 512], F32, tag="proj",
                                                name=f"ppv_{c}_{mt}")
                                msl = slice(mt * KTS, (mt + 1) * KTS)
                                for it in range(NT):
                                    nc.tensor.matmul(
                                        ps_t[0:KTS, :], acts[it][:, msl], wvs[it][:],
                                        start=(it == 0), stop=False,
                                    )
                                nc.tensor.matmul(
                                    ps_t[0:KTS, :], ones_row[:],
                                    bv_row[:, c * 512:(c + 1) * 512],
                                    start=False, stop=True,
                                )
                                st_t = stp.tile([128, 512], BF16, tag="stage",
                                                name=f"stv_{c}_{mt}")
                                nc.vector.tensor_copy(st_t[0:KTS, :], ps_t[0:KTS, :])
                                b, r = divmod(mt, KT)
                                nc.sync.dma_start(
                                    od[b, r * KTS:(r + 1) * KTS, c * 512:(c + 1) * 512],
                                    st_t[0:KTS, :],
                                )

        # ---- phase C: attention (head pairs) --------------------------
        with ExitStack() as actx:
            wfp = actx.enter_context(tc.tile_pool(name="wfp", bufs=8))
            hp = actx.enter_context(tc.tile_pool(name="hp", bufs=2))
            vp = actx.enter_context(tc.tile_pool(name="vp", bufs=4))
            ep = actx.enter_context(tc.tile_pool(name="ep", bufs=3))
            ip = actx.enter_context(tc.tile_pool(name="ip", bufs=3))
            ctxp = actx.enter_context(tc.tile_pool(name="ctxp", bufs=2 * BP * NT))
            tap = actx.enter_context(tc.tile_pool(name="tap", bufs=4 * KT + 2))
            fp = actx.enter_context(tc.tile_pool(name="fp", bufs=2))

            wf8 = []
            for it in range(NT):
                wf_t = wfp.tile([128, D], BF16, tag="wf", name=f"wf_{it}")
                nc.sync.dma_start(wf_t[:], Wf_d.ap()[it * 128:(it + 1) * 128, :])
                wf8.append(wf_t)

            ctx_tiles = {}
            ta_saved = {}
            for b in range(BP):
                for ri in range(2):
                    ctx_tiles[(b, ri)] = [
                        ctxp.tile([128, L], BF16, tag="ctx", name=f"ctx{b}_{ri}_{it}")
                        for it in range(NT)
                    ]
                    ta_saved[(b, ri)] = []

            for b in range(BP):
                for pr in range(H // 2):  # head pairs
                    psl = slice(pr * 128, (pr + 1) * 128)
                    k2r = hp.tile([128, L], BF16, tag="k2r", name=f"k2r{b}_{pr}")
                    nc.sync.dma_start(k2r[:], kr_d[b, psl, :])
                    k2i = hp.tile([128, L], BF16, tag="k2i", name=f"k2i{b}_{pr}")
                    nc.sync.dma_start(k2i[:], ki_d[b, psl, :])
                    q2r = hp.tile([128, L], BF16, tag="q2r", name=f"q2r{b}_{pr}")
                    nc.sync.dma_start(q2r[:], qr_d[b, psl, :])
                    q2i = hp.tile([128, L], BF16, tag="q2i", name=f"q2i{b}_{pr}")
                    nc.sync.dma_start(q2i[:], qi_d[b, psl, :])
                    q2rn = hp.tile([128, L], BF16, tag="q2rn", name=f"q2rn{b}_{pr}")
                    nc.vector.tensor_scalar_mul(q2rn[:], q2r[:], -1.0)

                    # V tiles for both heads of the pair: [keys, kt, 2*(DH+1)]
                    v2 = {}
                    for ri, vd in ((0, vr_d), (1, vi_d)):
                        v_t = vp.tile([KTS, KT, 132], BF16, tag="v2",
                                      name=f"v2_{b}_{pr}_{ri}")
                        src = vd[b].rearrange("(kt p) d -> p kt d", p=KTS)
                        nc.sync.dma_start(
                            v_t[:, :, 0:DH], src[:, :, pr * 128:pr * 128 + DH])
                        nc.sync.dma_start(
                            v_t[:, :, 66:66 + DH],
                            src[:, :, pr * 128 + DH:(pr + 1) * 128])
                        nc.any.memset(v_t[:, :, DH:DH + 1], 1.0)
                        nc.any.memset(v_t[:, :, 130:131], 1.0)
                        v2[ri] = v_t

                    for ri, (ka, kb, qa, qb) in enumerate(
                        ((k2r, k2i, q2r, q2i), (k2r, k2i, q2i, q2rn))
                    ):
                        ex_tiles = []
                        for kt in range(KT):
                            ksl = slice(kt * KTS, (kt + 1) * KTS)
                            sc = psp.tile([KTS, 1024], F32, tag="sc",
                                          name=f"sc{b}_{pr}_{ri}_{kt}")
                            # two heads in disjoint PE row groups, concurrent
                            nc.tensor.matmul(sc[:, 0:L], ka[0:DH, ksl],
                                             qa[0:DH, :], start=True, stop=False)
                            nc.tensor.matmul(sc[:, 512:512 + L], ka[DH:128, ksl],
                                             qa[DH:128, :], start=True, stop=False)
                            nc.tensor.matmul(sc[:, 0:L], kb[0:DH, ksl],
                                             qb[0:DH, :], start=False, stop=True)
                            nc.tensor.matmul(sc[:, 512:512 + L], kb[DH:128, ksl],
                                             qb[DH:128, :], start=False, stop=True)
                            ex = ep.tile([KTS, 1012], BF16, tag="exp",
                                         name=f"ex{b}_{pr}_{ri}_{kt}")
                            nc.scalar.activation(ex[:], sc[:, 0:1012], AF.Exp,
                                                 scale=SCALE)
                            ex_tiles.append(ex)

                        for hh in range(2):
                            h = pr * 2 + hh
                            voff = hh * 66
                            eoff = hh * 512
                            pv = psp.tile([DH + 1, 512], F32, tag="pv",
                                          name=f"pv{b}_{h}_{ri}")
                            for kt in range(KT):
                                nc.tensor.matmul(
                                    pv[:, 0:L],
                                    v2[ri][:, kt, voff:voff + DH + 1],
                                    ex_tiles[kt][:, eoff:eoff + L],
                                    start=(kt == 0), stop=(kt == KT - 1),
                                )
                            invz = ip.tile([1, L], F32, tag="invz",
                                           name=f"iz{b}_{h}_{ri}")
                            nc.vector.reciprocal_approx_fast(
                                invz[:], pv[DH:DH + 1, 0:L])
                            invzb = ip.tile([DH, L], F32, tag="invzb",
                                            name=f"izb{b}_{h}_{ri}")
                            nc.gpsimd.partition_broadcast(invzb[:], invz[:])
                            po = (h % 2) * DH
                            nc.vector.tensor_mul(
                                ctx_tiles[(b, ri)][h // 2][po:po + DH, :],
                                pv[0:DH, 0:L], invzb[:],
                            )

                    if pr == 0:
                        # top_attn (head 0), natural [q, keys] orientation
                        for ri, (la, lb, ra, rb) in enumerate(
                            ((q2r, q2i, k2r, k2i), (q2i, q2rn, k2r, k2i))
                        ):
                            for qt in range(KT):
                                s0 = psp.tile([KTS, 512], F32, tag="pv",
                                              name=f"s0_{b}_{ri}_{qt}")
                                qsl = slice(qt * KTS, (qt + 1) * KTS)
                                nc.tensor.matmul(s0[:, 0:L], la[0:DH, qsl],
                                                 ra[0:DH, :], start=True, stop=False)
                                nc.tensor.matmul(s0[:, 0:L], lb[0:DH, qsl],
                                                 rb[0:DH, :], start=False, stop=True)
                                e0 = ip.tile([KTS, L], F32, tag="e0", bufs=4,
                                             name=f"e0_{b}_{ri}_{qt}")
                                z0 = ip.tile([KTS, 1], F32, tag="z0", bufs=4,
                                             name=f"z0_{b}_{ri}_{qt}")
                                nc.scalar.activation(e0[:], s0[:, 0:L], AF.Exp,
                                                     scale=SCALE, accum_out=z0[:])
                                iz0 = ip.tile([KTS, 1], F32, tag="iz0", bufs=4,
                                              name=f"iz0_{b}_{ri}_{qt}")
                                nc.vector.reciprocal_approx_fast(iz0[:], z0[:])
                                a0 = tap.tile([KTS, L], F32, tag="ta",
                                              name=f"a0_{b}_{ri}_{qt}")
                                nc.vector.tensor_scalar_mul(a0[:], e0[:], iz0[:])
                                ta_saved[(b, ri)].append(a0)

            # ---- final linear + magnitudes (both batches, grouped) -----
            for b in range(BP):
                for dt in range(NT):
                    dsl = slice(dt * 128, (dt + 1) * 128)
                    fo = {}
                    for ri in range(2):
                        ps_f = psp.tile([128, 512], F32, tag="proj",
                                        name=f"pf{b}_{ri}_{dt}")
                        for it in range(NT):
                            nc.tensor.matmul(
                                ps_f[:, 0:L], wf8[it][:, dsl],
                                ctx_tiles[(b, ri)][it][:],
                                start=(it == 0), stop=(it == NT - 1),
                            )
                        f_t = fp.tile([128, L], F32, tag=f"fo{ri}",
                                      name=f"fo{b}_{ri}_{dt}")
                        nc.scalar.activation(f_t[:], ps_f[:, 0:L], AF.Identity,
                                             bias=bias_tiles[("bf", dt)][:])
                        fo[ri] = f_t
                    m1 = fp.tile([128, L], F32, tag="m1", name=f"m1_{b}_{dt}")
                    nc.vector.tensor_mul(m1[:], fo[0][:], fo[0][:])
                    m2 = fp.tile([128, L], F32, tag="m2", name=f"m2_{b}_{dt}")
                    nc.vector.tensor_mul(m2[:], fo[1][:], fo[1][:])
                    nc.vector.tensor_add(m1[:], m1[:], m2[:])
                    mo = fp.tile([128, L], F32, tag="mo", name=f"mo_{b}_{dt}")
                    nc.scalar.activation(mo[:], m1[:], AF.Sqrt)
                    nc.sync.dma_start(outT_d.ap()[b, dsl, :], mo[:])

            for b in range(BP):
                for qt in range(KT):
                    a_r = ta_saved[(b, 0)][qt]
                    a_i = ta_saved[(b, 1)][qt]
                    t1 = fp.tile([KTS, L], F32, tag="t1", name=f"t1_{b}_{qt}")
                    nc.vector.tensor_mul(t1[:], a_r[:], a_r[:])
                    t2 = fp.tile([KTS, L], F32, tag="t2", name=f"t2_{b}_{qt}")
                    nc.vector.tensor_mul(t2[:], a_i[:], a_i[:])
                    nc.vector.tensor_add(t1[:], t1[:], t2[:])
                    to = fp.tile([KTS, L], F32, tag="to", name=f"to_{b}_{qt}")
                    nc.scalar.activation(to[:], t1[:], AF.Sqrt)
                    nc.sync.dma_start(ta_d.ap()[b, qt * KTS:(qt + 1) * KTS, :], to[:])


def _get_module():
    if "nc" not in _CACHE:
        _CACHE["nc"] = _build()
    return _CACHE["nc"]


def _run(inputs, trace=False):
    nc = _get_module()
    key = np.asarray(inputs["key"], np.float32)
    value = np.asarray(inputs["value"], np.float32)
    query = np.asarray(inputs["query"], np.float32)
    phT = np.ascontiguousarray(np.asarray(inputs["phase"], np.float32).T)
    Ws = {n: np.ascontiguousarray(np.asarray(inputs[n], np.float32))
          for n in ("Wk", "Wv", "Wq", "Wf")}
    Wf_bf = Ws["Wf"].astype(ml_dtypes.bfloat16)
    bs = {n: np.asarray(inputs[n], np.float32) for n in ("bk", "bv", "bq", "bf")}

    in_maps = []
    for c in range(NCORES):
        sl = slice(c * BP, (c + 1) * BP)
        in_maps.append({
            "kT": np.ascontiguousarray(key[sl].transpose(0, 2, 1)),
            "vT": np.ascontiguousarray(value[sl].transpose(0, 2, 1)),
            "qT": np.ascontiguousarray(query[sl].transpose(0, 2, 1)),
            "phT": phT,
            "Wk": Ws["Wk"], "Wv": Ws["Wv"], "Wq": Ws["Wq"], "Wf": Wf_bf,
            "bk": bs["bk"], "bv": bs["bv"], "bq": bs["bq"], "bf": bs["bf"],
        })

    res = run_bass_kernel_spmd(nc, in_maps, list(range(NCORES)), trace=trace)
    out = np.empty((B, L, D), np.float32)
    ta = np.empty((B, L, L), np.float32)
    for c in range(NCORES):
        sl = slice(c * BP, (c + 1) * BP)
        out[sl] = res.results[c]["outT"].transpose(0, 2, 1)
        ta[sl] = res.results[c]["ta"]
    return (out, ta), res


def kernel(**inputs):
    (out, ta), _ = _run(inputs, trace=False)
    return out, ta
